# revision 24
# baseline (speedup 1.0000x reference)
"""Trainium2 Bass kernel for nn_AdapterBlock (LN -> dwconv x3 -> SE -> residual).

Data-parallel over batch: 8 samples -> 8 NeuronCores. Per core:
  - cast x f32->bf16 while DMA'ing into SBUF layout A [128 t_lo, 16 t_hi, 1024 c]
  - LayerNorm stats split between DVE bn_stats and ScalarE accum paths,
    Rsqrt on ScalarE, per-tile fused apply on DVE
  - layout A->B transpose on TensorE: per (t, ch) 128x128 matmul with the
    data as stationary and a bf16 identity as moving, batched PSUM
    evacuation spread across Scalar/GpSimd/DVE
  - conv1 (k=3): DVE, full-T tensor_scalar/tensor_tensor ops (4x/2x modes)
  - conv2*conv3 fused into one k=7 depthwise conv on TensorE (host-composed
    weights); SAME-boundary mismatch fixed by 2 extra accumulate-matmuls per
    edge; SE global-average pool rides the evacuation's accum_out
  - SE MLP on TensorE, sigmoid gate on ScalarE
  - transpose B->A on TensorE with diag(gate) as the moving operand (gate
    applied for free) paired with an identity matmul that accumulates the
    residual into the same PSUM; ScalarE copy-evacuation; f32 stores on the
    sync HW-DGE ring
"""

import os
import sys

sys.path.insert(0, "/opt/trn_rl_repo")

import numpy as np

import concourse.bass as bass  # noqa: F401
import concourse.bacc as bacc
import concourse.tile as tile
import concourse.mybir as mybir
from concourse.bass_utils import run_bass_kernel_spmd

B, T, C = 8, 2048, 1024
N_CORES = 8
NT = T // 128          # 16 t-tiles
NCH = C // 128         # 8 channel groups
H = C // 16            # SE hidden = 64
PAD = 4                # zero pad each side of the time axis (>= conv halo 3)
TF = T + 2 * PAD
HT = T // 2            # half-tile free size for PSUM (2 banks)
EPS = 1e-5

F32 = mybir.dt.float32
BF16 = mybir.dt.bfloat16
AF = mybir.ActivationFunctionType
OP = mybir.AluOpType

# --- tunables ------------------------------------------------------------
N_STATS_ACT = 4 * (int(os.environ.get("K_STATS_ACT", "4")) // 4)  # first N t-tiles on ScalarE
EVAC_PAT = os.environ.get("K_EVAC", "s")              # head evac engines
TAIL_PAT = os.environ.get("K_TAIL", "sd")              # tail evac engines

_CACHE = {}


def _build():
    nc = bacc.Bacc("TRN2", target_bir_lowering=False, debug=False,
                   num_devices=N_CORES)

    x_ext = nc.dram_tensor("x", [T, C], F32, kind="ExternalInput").ap()
    res_ext = nc.dram_tensor("res", [T, C], F32, kind="ExternalInput").ap()
    w1_ext = nc.dram_tensor("w1p", [128, NCH, 3], F32, kind="ExternalInput").ap()
    b1_ext = nc.dram_tensor("b1p", [128, NCH], F32, kind="ExternalInput").ap()
    fc1_ext = nc.dram_tensor("fc1p", [128, NCH, H], F32, kind="ExternalInput").ap()
    fc2_ext = nc.dram_tensor("fc2p", [H, NCH, 128], F32, kind="ExternalInput").ap()
    d23_ext = nc.dram_tensor("d23", [128, NCH, 7, 128], BF16, kind="ExternalInput").ap()
    dec_ext = nc.dram_tensor("dec", [128, NCH, 4, 128], BF16, kind="ExternalInput").ap()
    ws_ext = nc.dram_tensor("wsump", [128, NCH], F32, kind="ExternalInput").ap()
    ce_ext = nc.dram_tensor("cep", [128, NCH, 6], F32, kind="ExternalInput").ap()
    id_ext = nc.dram_tensor("ident", [128, 128], BF16, kind="ExternalInput").ap()
    out_ext = nc.dram_tensor("out", [T, C], F32, kind="ExternalOutput").ap()

    x_src = x_ext.rearrange("(th p) c -> p th c", p=128)
    res_src = res_ext.rearrange("(th p) c -> p th c", p=128)
    out_dst = out_ext.rearrange("(th p) c -> p th c", p=128)

    with tile.TileContext(nc) as tc:
        with tc.tile_pool(name="main", bufs=1) as pool:
            _body(nc, tc, pool,
                  x_src, res_src, out_dst,
                  w1_ext, b1_ext, fc1_ext, fc2_ext, d23_ext, dec_ext, id_ext,
                  ws_ext, ce_ext)

    nc.compile()
    return nc


def _body(nc, tc, pool, x_src, res_src, out_dst,
          w1_ext, b1_ext, fc1_ext, fc2_ext, d23_ext, dec_ext, id_ext,
          ws_ext, ce_ext):
    # ---- weights in ----
    w1sb = pool.tile([128, NCH, 3], F32, tag="w1sb")
    b1sb = pool.tile([128, NCH], F32, tag="b1sb")
    fc1sb = pool.tile([128, NCH, H], F32, tag="fc1sb")
    fc2sb = pool.tile([H, NCH, 128], F32, tag="fc2sb")
    d23sb = pool.tile([128, NCH, 7, 128], BF16, tag="d23sb")
    decsb = pool.tile([128, NCH, 4, 128], BF16, tag="decsb")
    isb = pool.tile([128, 128], BF16, tag="isb")
    nc.sync.dma_start(isb[:], id_ext)
    nc.sync.dma_start(w1sb[:], w1_ext)
    nc.sync.dma_start(b1sb[:], b1_ext)
    nc.sync.dma_start(fc1sb[:], fc1_ext)
    nc.sync.dma_start(fc2sb[:], fc2_ext)

    # ---- head: load -> LN stats -> apply -> PE transpose -> evac ----
    NA = N_STATS_ACT
    zX = pool.tile([128, NT, C], BF16, tag="zX")
    muvar = pool.tile([128, NT, 2], F32, tag="muvar")
    sums = pool.tile([128, NT], F32, tag="sums")
    sumsq = pool.tile([128, NT], F32, tag="sumsq")
    musq = pool.tile([128, 4], F32, tag="musq")
    scr = pool.tile([128, C], BF16, tag="scr")
    scr2 = pool.tile([128, C], BF16, tag="scr2")
    rstd = pool.tile([128, NT], F32, tag="rstd")
    epsb = pool.tile([128, 1], F32, tag="epsb")
    nc.vector.memset(epsb[:], EPS)
    xB = pool.tile([128, NCH, TF], BF16, tag="xB")
    nc.vector.memset(xB[:, :, 0:PAD], 0.0)
    nc.vector.memset(xB[:, :, PAD + T:TF], 0.0)

    evac_eng = {"s": nc.scalar, "g": nc.gpsimd, "d": nc.vector}

    with tc.tile_pool(name="hps", bufs=1, space="PSUM") as hpool:
        for g in range(4):
            ts0 = 4 * g
            if g == 3:
                # diag stacks ride the gpsimd ring between the g2 and g3
                # x loads: x owns the queues first, d23 lands before conv23
                for ch in range(NCH):
                    nc.gpsimd.dma_start(d23sb[:, ch, :, :],
                                        d23_ext[:, ch, :, :])
                nc.gpsimd.dma_start(decsb[:], dec_ext)
            nc.gpsimd.dma_start(zX[:, ts0:ts0 + 2, :],
                                x_src[:, ts0:ts0 + 2, :])
            nc.gpsimd.dma_start(zX[:, ts0 + 2:ts0 + 4, :],
                                x_src[:, ts0 + 2:ts0 + 4, :])
            for t in range(ts0, ts0 + 4):
                if t < NA:
                    nc.scalar.activation(scr[:], zX[:, t, :], AF.Copy,
                                         accum_out=sums[:, t:t + 1])
                    nc.scalar.activation(scr2[:], zX[:, t, :], AF.Square,
                                         accum_out=sumsq[:, t:t + 1])
                else:
                    bs = pool.tile([128, 2, 6], F32, tag="bstats",
                                   name=f"bs_{t}", bufs=2)
                    nc.vector.bn_stats(bs[:, 0, :], zX[:, t, 0:512])
                    nc.vector.bn_stats(bs[:, 1, :], zX[:, t, 512:1024])
                    nc.vector.bn_aggr(muvar[:, t, :], bs[:])
            gs = slice(ts0, ts0 + 4)
            if ts0 < NA:  # ACT-stats tiles in this group
                nc.vector.tensor_scalar_mul(muvar[:, gs, 0:1], sums[:, gs],
                                            1.0 / C)
                nc.vector.tensor_tensor(musq[:], muvar[:, gs, 0:1],
                                        muvar[:, gs, 0:1], op=OP.mult)
                nc.vector.scalar_tensor_tensor(muvar[:, gs, 1:2],
                                               sumsq[:, gs], 1.0 / C,
                                               musq[:], OP.mult, OP.subtract)
            nc.scalar.activation(rstd[:, gs], muvar[:, gs, 1:2], AF.Sqrt,
                                 bias=epsb[:])
            nc.vector.reciprocal(rstd[:, gs], rstd[:, gs])
            for t in range(ts0, ts0 + 4):
                nc.vector.tensor_scalar(zX[:, t, :], zX[:, t, :],
                                        muvar[:, t, 0:1], rstd[:, t:t + 1],
                                        OP.subtract, OP.mult)
                # PE transpose: data stationary, identity moving
                psA = hpool.tile([128, NCH, 128], F32, tag="psA",
                                 name=f"psA_{t}", bufs=2)
                for ch in range(NCH):
                    nc.tensor.matmul(psA[:, ch, :],
                                     zX[:, t, ch * 128:(ch + 1) * 128],
                                     isb[:], start=True, stop=True)
                dst = xB[:, :, PAD + t * 128:PAD + (t + 1) * 128]
                ek = EVAC_PAT[t % len(EVAC_PAT)]
                if ek == "s":
                    nc.scalar.activation(dst, psA[:], AF.Copy)
                else:
                    evac_eng[ek].tensor_copy(dst, psA[:])

        # residual in: reuse zX (consumed by the transposes above); overlaps
        # the conv phase entirely.
        for q in range(4):
            nc.gpsimd.dma_start(zX[:, q * 4:(q + 1) * 4, :],
                                res_src[:, q * 4:(q + 1) * 4, :])

        # ---- convs: two half-T sweeps so sweep 0 overlaps the head ----
        rF = pool.tile([128, NCH, TF], BF16, tag="rF")
        seam = HT + 4
        acc = pool.tile([128, seam], BF16, tag="acc")
        tmp1 = pool.tile([128, seam], BF16, tag="tmp1")
        tmp2 = pool.tile([128, seam], BF16, tag="tmp2")
        nc.vector.memset(rF[:, :, 0:PAD], 0.0)
        nc.vector.memset(rF[:, :, PAD + T:TF], 0.0)
        c3 = pool.tile([128, NCH, TF], BF16, tag="c3")
        nc.vector.memset(c3[:, :, 0:PAD], 0.0)
        nc.vector.memset(c3[:, :, PAD + T:TF], 0.0)
        pools = pool.tile([128, NCH, 2], F32, tag="pools")
        gate = pool.tile([128, NCH], F32, tag="gate")
        dg = pool.tile([128, NCH, 128], BF16, tag="dg")

        with tc.tile_pool(name="ps", bufs=2, space="PSUM") as psum:
            for hh in range(2):
                lo, hi = (0, seam) if hh == 0 else (seam, T)
                base = hh * HT
                for ch in range(NCH):

                    def xsw(d, ch=ch, lo=lo, hi=hi):
                        return xB[:, ch, PAD + lo + d:PAD + hi + d]

                    # conv1: k=3 on DVE (TS at 4x, TT at 2x)
                    n = hi - lo
                    nc.vector.tensor_scalar(acc[:, :n], xsw(-1),
                                            w1sb[:, ch, 0:1],
                                            b1sb[:, ch:ch + 1],
                                            OP.mult, OP.add)
                    nc.vector.tensor_scalar_mul(tmp1[:, :n], xsw(0),
                                                w1sb[:, ch, 1:2])
                    nc.vector.tensor_tensor(acc[:, :n], acc[:, :n],
                                            tmp1[:, :n], op=OP.add)
                    nc.vector.tensor_scalar_mul(tmp2[:, :n], xsw(1),
                                                w1sb[:, ch, 2:3])
                    nc.vector.tensor_tensor(acc[:, :n], acc[:, :n],
                                            tmp2[:, :n], op=OP.add)
                    nc.vector.tensor_scalar_max(
                        rF[:, ch, PAD + lo:PAD + hi], acc[:, :n], 0.0)

                    # fused conv23: k=7, shifts -3..3, plus SAME-boundary fix
                    ps2 = psum.tile([128, HT], F32, tag="cps",
                                    name=f"c23ps_{ch}_{hh}")
                    for k in range(7):
                        for q in range(2):
                            off = PAD - 3 + k + base + q * 512
                            edge_q = (hh == 0 and q == 0) or \
                                     (hh == 1 and q == 1)
                            nc.tensor.matmul(ps2[:, q * 512:(q + 1) * 512],
                                             d23sb[:, ch, k, :],
                                             rF[:, ch, off:off + 512],
                                             start=(k == 0),
                                             stop=(k == 6 and not edge_q))
                    if hh == 0:
                        # out[0] -= w3[0]*(w2[3] r[0] + w2[4] r[1])
                        nc.tensor.matmul(ps2[:, 0:1], decsb[:, ch, 0, :],
                                         rF[:, ch, PAD:PAD + 1],
                                         start=False, stop=False)
                        nc.tensor.matmul(ps2[:, 0:1], decsb[:, ch, 1, :],
                                         rF[:, ch, PAD + 1:PAD + 2],
                                         start=False, stop=True)
                    else:
                        # out[T-1] -= w3[2]*(w2[0] r[T-2] + w2[1] r[T-1])
                        nc.tensor.matmul(ps2[:, HT - 1:HT],
                                         decsb[:, ch, 2, :],
                                         rF[:, ch, PAD + T - 2:PAD + T - 1],
                                         start=False, stop=False)
                        nc.tensor.matmul(ps2[:, HT - 1:HT],
                                         decsb[:, ch, 3, :],
                                         rF[:, ch, PAD + T - 1:PAD + T],
                                         start=False, stop=True)
                    nc.scalar.activation(
                        c3[:, ch, PAD + base:PAD + base + HT],
                        ps2[:], AF.Copy,
                        accum_out=pools[:, ch, hh:hh + 1])

    # ---- SE MLP ----
    with tc.tile_pool(name="seps", bufs=1, space="PSUM") as se_ps:
        h_ps = se_ps.tile([H, 2], F32, tag="hps")
        for ch in range(NCH):
            nc.tensor.matmul(h_ps[:], fc1sb[:, ch, :], pools[:, ch, :],
                             start=(ch == 0), stop=(ch == NCH - 1))
        h_half = pool.tile([H, 2], F32, tag="h_half")
        nc.scalar.activation(h_half[:], h_ps[:], AF.Relu)
        h_sb = pool.tile([H, 1], F32, tag="hsb")
        nc.vector.tensor_tensor(h_sb[:], h_half[:, 0:1], h_half[:, 1:2],
                                op=OP.add)
        g_ps = se_ps.tile([128, NCH], F32, tag="gps")
        for ch in range(NCH):
            nc.tensor.matmul(g_ps[:, ch:ch + 1], fc2sb[:, ch, :], h_sb[:],
                             start=True, stop=True)
        nc.scalar.activation(gate[:], g_ps[:], AF.Sigmoid)
        # per-group diag(gate): identity rows scaled by the per-partition gate
        for ch in range(NCH):
            nc.vector.tensor_scalar_mul(dg[:, ch, :], isb[:],
                                        gate[:, ch:ch + 1])

    # ---- tail: gated PE transpose back (+ residual matmul) + store ----
    outring = pool.tile([128, 4, C], F32, tag="outring")
    with tc.tile_pool(name="tps", bufs=1, space="PSUM") as tpool:
        for th in range(NT):
            psT = tpool.tile([128, NCH, 128], F32, tag="psT",
                             name=f"psT_{th}", bufs=3)
            for ch in range(NCH):
                nc.tensor.matmul(
                    psT[:, ch, :],
                    c3[:, ch, PAD + th * 128:PAD + (th + 1) * 128],
                    dg[:, ch, :], start=True, stop=False)
                nc.tensor.matmul(psT[:, ch, :], isb[:],
                                 zX[:, th, ch * 128:(ch + 1) * 128],
                                 start=False, stop=True)
            # halve evac latency: Scalar and DVE each evacuate half the tile
            nc.scalar.activation(
                outring[:, th % 4, 0:C // 2],
                psT[:, 0:NCH // 2, :].rearrange("p a b -> p (a b)"),
                AF.Copy)
            nc.vector.tensor_copy(
                outring[:, th % 4, C // 2:C],
                psT[:, NCH // 2:NCH, :].rearrange("p a b -> p (a b)"))
            nc.sync.dma_start(out_dst[:, th, :], outring[:, th % 4, :])


def _prep_weights(ln_w, ln_b, w1, w2, w3, fc1, fc2):
    import ml_dtypes
    w1 = w1[:, 0, :].astype(np.float64)   # [C, 3]
    w2 = w2[:, 0, :].astype(np.float64)   # [C, 5]
    w3 = w3[:, 0, :].astype(np.float64)   # [C, 3]
    ln_w = ln_w.astype(np.float64)
    ln_b = ln_b.astype(np.float64)
    w1f = w1 * ln_w[:, None]
    b1 = (ln_b * w1.sum(axis=1))

    def to_plh(a):  # [C, K] -> [128, NCH, K]
        return np.ascontiguousarray(
            a.reshape(NCH, 128, -1).transpose(1, 0, 2)).astype(np.float32)

    w1p = to_plh(w1f)
    b1p = np.ascontiguousarray(b1.reshape(NCH, 128).T).astype(np.float32)
    fc1p = to_plh((fc1.astype(np.float64) / T).T)
    fc2p = np.ascontiguousarray(
        fc2.astype(np.float64).T.reshape(H, NCH, 128)).astype(np.float32)

    w23 = np.stack([np.convolve(w3[c], w2[c]) for c in range(C)])  # [C, 7]
    # edge-fix coefficients (negated: they accumulate into the psum)
    ec = np.stack([-w3[:, 0] * w2[:, 3], -w3[:, 0] * w2[:, 4],
                   -w3[:, 2] * w2[:, 0], -w3[:, 2] * w2[:, 1]], axis=1)  # [C,4]

    def diags(wk):  # [C, K] -> [128, NCH, K, 128] bf16 (ch-major)
        K = wk.shape[1]
        d = np.zeros((K, NCH, 128, 128), np.float32)
        for k in range(K):
            for chh in range(NCH):
                np.fill_diagonal(d[k, chh], wk[chh * 128:(chh + 1) * 128, k])
        return np.ascontiguousarray(
            d.transpose(2, 1, 0, 3)).astype(ml_dtypes.bfloat16)

    ident = np.eye(128, dtype=np.float32).astype(ml_dtypes.bfloat16)

    # pool-shortcut coefficients: pool(conv23(r)) = wsum*S + sum ce_j*edge_j
    # edge order matches redge: [r0, r1, r2, r(T-3), r(T-2), r(T-1)]
    wsum = w23.sum(axis=1)  # [C]
    ce = np.stack([
        -(w23[:, 4] + w23[:, 5] + w23[:, 6]) + ec[:, 0],
        -(w23[:, 5] + w23[:, 6]) + ec[:, 1],
        -w23[:, 6],
        -w23[:, 0],
        -(w23[:, 0] + w23[:, 1]) + ec[:, 2],
        -(w23[:, 0] + w23[:, 1] + w23[:, 2]) + ec[:, 3],
    ], axis=1)  # [C, 6]
    wsump = np.ascontiguousarray(wsum.reshape(NCH, 128).T).astype(np.float32)
    cep = to_plh(ce)

    return {"w1p": w1p, "b1p": b1p, "fc1p": fc1p, "fc2p": fc2p,
            "d23": diags(w23), "dec": diags(ec), "ident": ident,
            "wsump": wsump, "cep": cep}


def kernel(x, residual_input, ln_w, ln_b, w1, w2, w3, fc1, fc2):
    x = np.asarray(x, dtype=np.float32)
    residual_input = np.asarray(residual_input, dtype=np.float32)
    wts = _prep_weights(np.asarray(ln_w), np.asarray(ln_b),
                        np.asarray(w1), np.asarray(w2), np.asarray(w3),
                        np.asarray(fc1), np.asarray(fc2))

    if "nc" not in _CACHE:
        _CACHE["nc"] = _build()
    nc = _CACHE["nc"]

    in_maps = []
    for b in range(B):
        m = {"x": np.ascontiguousarray(x[b]),
             "res": np.ascontiguousarray(residual_input[b])}
        m.update(wts)
        in_maps.append(m)
    res = run_bass_kernel_spmd(nc, in_maps, core_ids=list(range(N_CORES)))
    out = np.stack([res.results[i]["out"] for i in range(N_CORES)], axis=0)
    return out.astype(np.float32)
